# revision 86
# baseline (speedup 1.0000x reference)
"""Trainium2 Bass kernel for prefix-attention block (B=8,T=1024,C=1024,H=16,Tp=64).

Strategy: data-parallel over batch B across 8 NeuronCores (one batch element
per core, no collectives). Single fused software-pipelined schedule:

  - Inputs land via ~10 large multi-dim-AP DMAs (fast Sync issue, queues
    overlap with early compute).
  - Projections (q/k/v/prefix) are computed just-in-time and interleaved as
    "filler" PE work between attention score/AV blocks, so the TensorE never
    idles while ScalarE (exp) paces the attention inner loop.
  - Stage order: all (pair, ir=0) stages (queries 0:512), then all ir=1
    (queries 512:1024). Output projection for the first query half overlaps
    the second half's attention.
  - Per-stage combine: softmax sums come out of the AV matmul via a ones
    column; reciprocals are broadcast across partitions with a step-0-AP
    SBUF->SBUF DMA; normalization multiplies read the PSUM accumulators
    directly (no extraction copies, no tail combine phase).
"""

import numpy as np
import ml_dtypes

B, T, C, H, D, TP = 8, 1024, 1024, 16, 64, 64
NT = T // 128   # 8 token tiles
KC = C // 128   # 8 contraction chunks

_CACHE = {}


def _emit(nc, tc, dram, debug=False):
    import concourse.bass as bass
    import concourse.mybir as mybir
    from contextlib import ExitStack
    from concourse.tile_rust import add_dep_helper

    BF = mybir.dt.bfloat16
    F32 = mybir.dt.float32
    Exp = mybir.ActivationFunctionType.Exp

    pe_prev = [None]

    def pe_chain(inst):
        if pe_prev[0] is not None:
            add_dep_helper(inst.ins, pe_prev[0].ins, sync=False,
                           reason="forced PE order")
        pe_prev[0] = inst

    with ExitStack() as top:
        top.enter_context(nc.allow_low_precision(
            reason="bf16 compute is intentional; f32 PSUM accumulation"))
        persist = top.enter_context(tc.tile_pool(name="persist", bufs=1))
        ps_gen = top.enter_context(tc.tile_pool(name="ps_gen", bufs=2, space="PSUM"))
        ps_acc = top.enter_context(tc.tile_pool(name="ps_acc", bufs=2, space="PSUM"))

        # ---- persistent SBUF ----
        xT = persist.tile([128, KC * T], BF, tag="xT", name="xT")
        wqk = persist.tile([128, 16 * C], BF, tag="wqk", name="wqk")
        wv = persist.tile([128, KC * C], BF, tag="wv", name="wv")
        wp = persist.tile([128, 8 * C], BF, tag="wp", name="wp")
        pT = persist.tile([128, KC * TP], BF, tag="pT", name="pT")
        qkT = [persist.tile([128, T], BF, tag=f"qkT{m}", name=f"qkT{m}")
               for m in range(16)]
        vsb = [persist.tile([128, H * 65], BF, tag=f"vsb{t}", name=f"vsb{t}")
               for t in range(NT)]
        kpT = [persist.tile([128, TP], BF, tag=f"kpT{m}", name=f"kpT{m}")
               for m in range(8)]
        vpsb = persist.tile([128, H * 65], BF, tag="vpsb", name="vpsb")
        masksb = persist.tile([128, 128], BF, tag="masksb", name="masksb")
        maskpsb = persist.tile([128, 64], BF, tag="maskpsb", name="maskpsb")
        yT = [persist.tile([128, T], BF, tag=f"yT{t}", name=f"yT{t}")
              for t in range(NT)]

        # ---- input DMAs (few big ones; issue order = need order) ----
        nc.sync.dma_start(out=masksb, in_=dram["mask"].ap())
        nc.sync.dma_start(out=maskpsb, in_=dram["maskp"].ap())
        nc.sync.dma_start(out=pT.rearrange("p (k t) -> p k t", t=TP),
                          in_=dram["pT"].ap().rearrange("(k p) t -> p k t", p=128))

        def w_sliced_dma(dst, dram_t, col0, m_dst):
            # dst[:, (m_dst*KC + k)*128 : +128] = W[k*128:(k+1)*128, col0 : +128]
            dt = dram_t.ap()
            src = bass.AP(tensor=dt.tensor, offset=dt.offset + col0,
                          ap=[[dt.ap[0][0], 128],          # p within chunk
                              [dt.ap[0][0] * 128, KC],     # k
                              [1, 128]])                   # col
            dstv = bass.AP(tensor=dst.tensor,
                           offset=dst.offset + m_dst * KC * 128,
                           ap=[[dst.ap[0][0], 128],
                               [128, KC],
                               [1, 128]])
            nc.sync.dma_start(out=dstv, in_=src)

        wkp = persist.tile([128, 8 * C], BF, tag="wkp", name="wkp")
        pwvp = ExitStack()
        wvp = pwvp.enter_context(tc.tile_pool(name="pwvp", bufs=1)).tile(
            [128, KC * C], BF, tag="wvp", name="wvp")

        for m in range(2):
            w_sliced_dma(wkp, dram["wkp"], m * 128, m)
        for k2 in range(4):
            nc.sync.dma_start(
                out=xT.rearrange("p (k t) -> p k t", t=T)[:, 2 * k2:2 * k2 + 2, :],
                in_=dram["xT"].ap().rearrange("(k p) t -> p k t", p=128)
                [:, 2 * k2:2 * k2 + 2, :])
        w_sliced_dma(wqk, dram["wqk"], 0, 0)            # q pair 0
        w_sliced_dma(wqk, dram["wqk"], C, 8)            # k pair 0
        nc.sync.dma_start(out=wvp.rearrange("p (k c) -> p k c", c=C),
                          in_=dram["wvp"].ap().rearrange("(k p) c -> p k c", p=128))
        w_sliced_dma(wqk, dram["wqk"], 128, 1)          # q pair 1
        w_sliced_dma(wqk, dram["wqk"], C + 128, 9)      # k pair 1
        for k2 in range(2):
            nc.sync.dma_start(
                out=wv.rearrange("p (k c) -> p k c", c=C)[:, 4 * k2:4 * k2 + 4, :],
                in_=dram["wv"].ap().rearrange("(k p) c -> p k c", p=128)
                [:, 4 * k2:4 * k2 + 4, :])
        for m in range(2, 8):
            w_sliced_dma(wkp, dram["wkp"], m * 128, m)
            w_sliced_dma(wqk, dram["wqk"], m * 128, m)
            w_sliced_dma(wqk, dram["wqk"], C + m * 128, 8 + m)
        for m in range(8):
            w_sliced_dma(wp, dram["wp"], m * 128, m)

        def wqk_s(m, k):
            return wqk[:, (m * KC + k) * 128:(m * KC + k) * 128 + 128]

        def wkp_s(m, k):
            return wkp[:, (m * KC + k) * 128:(m * KC + k) * 128 + 128]

        def wp_s(m, k):
            return wp[:, (m * KC + k) * 128:(m * KC + k) * 128 + 128]

        def xT_s(k, sl=None):
            base = xT[:, k * T:(k + 1) * T]
            return base if sl is None else base[:, sl]

        def wv_s(k):
            return wv[:, k * C:(k + 1) * C]

        def wvp_s(k):
            return wvp[:, k * C:(k + 1) * C]

        def pT_s(k):
            return pT[:, k * TP:(k + 1) * TP]

        # ---- projection emitters (granular, for filler interleaving) ----
        def kpT_group(m):
            ps = ps_gen.tile([128, TP], F32, tag="ps_g", name="ps_g")
            for k in range(KC):
                pe_chain(nc.tensor.matmul(ps, wkp_s(m, k), pT_s(k),
                                          start=(k == 0), stop=(k == KC - 1)))
            nc.vector.tensor_copy(kpT[m], ps)

        def qk_half(m, hf, _box):
            ps = ps_gen.tile([128, 512], F32, tag="ps_g", name="ps_g")
            for k in range(KC):
                pe_chain(nc.tensor.matmul(
                    ps, wqk_s(m, k), xT_s(k)[:, hf * 512:(hf + 1) * 512],
                    start=(k == 0), stop=(k == KC - 1)))
            nc.vector.tensor_copy(qkT[m][:, hf * 512:(hf + 1) * 512], ps)

        def v_half(tt, hf, _box):
            ps = ps_gen.tile([128, 512], F32, tag="ps_g", name="ps_g")
            sl = slice(tt * 128, (tt + 1) * 128)
            for k in range(KC):
                pe_chain(nc.tensor.matmul(
                    ps, xT_s(k, sl), wv_s(k)[:, hf * 512:(hf + 1) * 512],
                    start=(k == 0), stop=(k == KC - 1)))
            nc.vector.tensor_copy(
                vsb[tt].rearrange("p (h e) -> p h e", e=65)
                [:, hf * 8:(hf + 1) * 8, 0:64],
                ps.rearrange("p (h e) -> p h e", e=64))
            if hf == 1:
                nc.vector.memset(
                    vsb[tt].rearrange("p (h e) -> p h e", e=65)[:, :, 64:65], 1.0)

        def vpsb_group():
            vpv = vpsb.rearrange("p (h e) -> p h e", e=65)
            for hf in range(2):
                ps = ps_gen.tile([64, 512], F32, tag="ps_g", name="ps_g")
                for k in range(KC):
                    pe_chain(nc.tensor.matmul(
                        ps, pT_s(k), wvp_s(k)[:, hf * 512:(hf + 1) * 512],
                        start=(k == 0), stop=(k == KC - 1)))
                nc.vector.tensor_copy(vpv[0:64, hf * 8:(hf + 1) * 8, 0:64],
                                      ps.rearrange("p (h e) -> p h e", e=64))
                nc.vector.tensor_copy(vpv[64:128, hf * 8:(hf + 1) * 8, 0:64],
                                      ps.rearrange("p (h e) -> p h e", e=64))
            nc.vector.memset(vpv[:, :, 64:65], 1.0)

        # outproj: chunk (hf, m) = sum_k wp[k,m-slice].T @ yT[k][:, hf*512:...]
        def outproj_half(hf, m, half, ps_box):
            if half == 0:
                ps_box[0] = ps_gen.tile([128, 512], F32, tag="ps_g", name="ps_g")
            ps = ps_box[0]
            for k in range(4 * half, 4 * half + 4):
                pe_chain(nc.tensor.matmul(
                    ps, wp_s(m, k), yT[k][:, hf * 512:(hf + 1) * 512],
                    start=(k == 0), stop=(k == KC - 1)))
            if half == 1:
                stg = pstg.tile([128, 512], F32, tag="stg", name="stg")
                if m % 2:
                    nc.vector.tensor_copy(stg, ps)
                else:
                    nc.scalar.copy(stg, ps)
                nc.sync.dma_start(
                    out=dram["out"].ap()[m * 128:(m + 1) * 128,
                                         hf * 512:(hf + 1) * 512],
                    in_=stg)
                ps_box[0] = None

        # ---- filler queue ----
        # Ordered list of (key, closure) emitted into PE idle slots; before a
        # stage starts, everything tagged with its key is force-drained so the
        # forced PE order can never deadlock against a data dependency.
        fillers = []

        def add_group(key, fn, nargs):
            box = [None]
            fillers.append((key, lambda: fn(*nargs, 0, box)))
            fillers.append((key, lambda: fn(*nargs, 1, box)))

        # pre-loop leaves these to fillers: kpT 2..7, qk pairs 2..7, v tt4..7
        for p in range(2, 8):
            fillers.append(((p, 0), lambda m=p: kpT_group(m)))
            add_group((p, 0), qk_half, (p,))
            add_group((p, 0), qk_half, (8 + p,))
            if p - 2 < 4:
                add_group((0, 1), v_half, (p + 2,))
        for m in range(8):
            add_group("op0", outproj_half, (0, m))

        def pull_filler(allow_op0):
            while fillers:
                key, f = fillers[0]
                if key == "op0" and not allow_op0:
                    return False
                fillers.pop(0)
                f()
                return True
            return False

        def drain_until(stage_key):
            while any(k == stage_key for k, _ in fillers):
                key, f = fillers.pop(0)
                f()

        # select matrix for the recip-broadcast matmuls: one K=33 matmul maps
        # scratch row 32 (A recips) -> out rows 0:64 and row 0 (B) -> 64:128
        sel = persist.tile([33, 128], BF, tag="sel", name="sel")
        nc.vector.memset(sel, 0.0)
        nc.vector.memset(sel[32:33, 0:64], 1.0)
        nc.vector.memset(sel[0:1, 64:128], 1.0)

        class Stage:
            def __init__(self, p, ir):
                self.p, self.ir = p, ir
                self.i0 = ir * 512
                self.jmax = 4 * (ir + 1)
                self.qt, self.kt, self.kpt = qkT[p], qkT[8 + p], kpT[p]
                self.s_all, self.e_all = {}, {}

            def scores(self, jb):
                c0 = max(0, jb - 4 * self.ir) * 128
                st = ps_gen.tile([128, 1024], F32, tag="ps_g", name="ps_g")
                for hh, pb in enumerate((0, 64)):
                    pe_chain(nc.tensor.matmul(
                        st[:, hh * 512 + c0:hh * 512 + 512],
                        self.kt[pb:pb + 64, jb * 128:(jb + 1) * 128],
                        self.qt[pb:pb + 64, self.i0 + c0:self.i0 + 512],
                        start=True, stop=True))
                self.s_all[jb] = st

            def exps(self, jb):
                c0 = max(0, jb - 4 * self.ir) * 128
                st = self.s_all.pop(jb)
                et = pexp.tile([128, 1024], BF, tag="et", name="et")
                nc.scalar.activation(
                    et.rearrange("p (g n) -> p g n", g=2)[:, :, c0:512],
                    st.rearrange("p (g n) -> p g n", g=2)[:, :, c0:512],
                    Exp, scale=0.125)
                if jb >= 4 * self.ir:
                    dv = et.rearrange("p (g n) -> p g n", g=2)[:, :, c0:c0 + 128]
                    nc.gpsimd.tensor_mul(
                        dv, dv,
                        bass.AP(tensor=masksb.tensor, offset=masksb.offset,
                                ap=[list(masksb.ap[0]), [0, 2],
                                    list(masksb.ap[1])]))
                self.e_all[jb] = et

            def avs(self, jb):
                c0 = max(0, jb - 4 * self.ir) * 128
                et = self.e_all.pop(jb)
                for hh in range(2):
                    h = 2 * self.p + hh
                    pe_chain(nc.tensor.matmul(
                        self.Ats[:, hh * 512 + c0:hh * 512 + 512],
                        vsb[jb][:, h * 65:(h + 1) * 65],
                        et[:, hh * 512 + c0:hh * 512 + 512],
                        start=(jb == 0), stop=(jb == self.jmax - 1),
                        skip_group_check=True))

            def front1(self):
                # prefix scores, both heads quadrant-packed into [128, 512]
                spt = ps_gen.tile([128, 512], F32, tag="ps_g", name="ps_g")
                for hh, pb in enumerate((0, 64)):
                    pe_chain(nc.tensor.matmul(
                        spt[pb:pb + 64, :],
                        self.kpt[pb:pb + 64, :],
                        self.qt[pb:pb + 64, self.i0:self.i0 + 512],
                        start=True, stop=True,
                        tile_position=(pb, pb)))
                self.scores(0)
                ep = pep.tile([128, 512], BF, tag="ep", name="ep")
                nc.scalar.activation(ep, spt, Exp, scale=0.125)
                if self.ir == 0:
                    nc.gpsimd.tensor_mul(ep[:, 0:64], ep[:, 0:64], maskpsb)
                if debug and self.p == 0 and self.ir == 0:
                    nc.sync.dma_start(out=dram["d_ep"].ap(), in_=ep)
                self.eps = ep
                self.exps(0)

            def front2(self):
                self.scores(1)
                self.Bts = ps_acc.tile([65, 1024], F32, tag="ps_a", name="ps_a")
                for hh, pb in enumerate((0, 64)):
                    h = 2 * self.p + hh
                    pe_chain(nc.tensor.matmul(
                        self.Bts[:, hh * 512:hh * 512 + 512],
                        vpsb[pb:pb + 64, h * 65:(h + 1) * 65],
                        self.eps[pb:pb + 64, :],
                        start=True, stop=True))
                self.exps(1)

            def front2b(self):
                # evacuate B early: unnormalized data to SBUF (DVE), sums row
                # to the recip scratch (ACT). Frees the Bts slot mid-stage.
                self.rs = prs.tile([33, 1024], F32, tag="rs", name="rs")
                nc.scalar.copy(self.rs[0:1, :], self.Bts[64:65, :])
                self.tB = ptb.tile([128, 1024], BF, tag="tB", name="tB")
                nc.vector.tensor_copy(self.tB[64:128, :], self.Bts[0:64, :])
                self.Ats = ps_acc.tile([65, 1024], F32, tag="ps_a", name="ps_a")

            def main(self, allow_op0, after_block=None):
                for jb0 in range(0, self.jmax, 2):
                    for jb in (jb0 + 2, jb0 + 3):
                        if jb < self.jmax:
                            self.scores(jb)
                    for jb in (jb0 + 2, jb0 + 3):
                        if jb < self.jmax:
                            self.exps(jb)
                    pull_filler(allow_op0)
                    if after_block is not None:
                        # prev stage's combine: its bc matmuls must precede
                        # avs(0) in the forced PE order (Ats slot rotation)
                        after_block()
                        after_block = None
                        pull_filler(allow_op0)  # PE cover for the evac+mul
                    for jb in (jb0, jb0 + 1):
                        if jb < self.jmax - 1:
                            self.avs(jb)

            def av_last(self):
                self.avs(self.jmax - 1)

            def combine_recip(self):
                # A sums -> scratch row 32 (ACT), batched reciprocal, cast.
                # Off the forced-PE path so the PE keeps streaming meanwhile.
                rs = self.rs
                nc.scalar.copy(rs[32:33, :], self.Ats[64:65, :])
                nc.vector.reciprocal_approx_fast(rs, rs)
                self.rsb = prs.tile([33, 1024], BF, tag="rsb", name="rsb")
                nc.vector.tensor_copy(self.rsb, rs)

            def combine(self):
                # K=1 ones-matmul broadcast into PSUM (A rows 0:64, B rows
                # 64:128), evac to bf16 SBUF, then normalize+sum into yT.
                rsb = self.rsb
                bc_ps = ps_gen.tile([128, 1024], F32, tag="ps_g", name="ps_g")
                for hh in range(2):
                    cs = slice(hh * 512, (hh + 1) * 512)
                    pe_chain(nc.tensor.matmul(          # A -> rows 0:64
                        bc_ps[0:64, cs], sel[32:33, 0:64], rsb[32:33, cs],
                        start=True, stop=True, tile_position=(32, 0)))
                    pe_chain(nc.tensor.matmul(          # B -> rows 64:128
                        bc_ps[64:128, cs], sel[0:1, 64:128], rsb[0:1, cs],
                        start=True, stop=True, tile_position=(0, 64)))
                bc = pbc.tile([128, 1024], BF, tag="bc", name="bc")
                nc.scalar.copy(bc, bc_ps)
                uA = pua.tile([64, 1024], BF, tag="uA", name="uA")
                nc.vector.tensor_mul(uA, self.Ats[0:64, :], bc[0:64, :])
                uB = pua.tile([64, 1024], BF, tag="uB", name="uB")
                nc.vector.tensor_mul(uB, self.tB[64:128, :], bc[64:128, :])
                if debug and self.p == 0 and self.ir == 0:
                    nc.sync.dma_start(out=dram["d_bcA"].ap(), in_=bc[0:64, :])
                    nc.sync.dma_start(out=dram["d_uA"].ap(), in_=uA)
                    nc.sync.dma_start(out=dram["d_tB"].ap(), in_=uB)
                    nc.sync.dma_start(out=dram["d_bcB"].ap(), in_=bc[64:128, :])
                sl = slice(self.i0, self.i0 + 512)
                nc.vector.tensor_add(yT[self.p][0:64, sl],
                                     uA[:, 0:512], uB[:, 0:512])
                nc.vector.tensor_add(yT[self.p][64:128, sl],
                                     uA[:, 512:1024], uB[:, 512:1024])

        # ---- pre-loop: minimum to start stage (0, ir=0) ----
        box = [None]
        kpT_group(0)
        kpT_group(1)
        qk_half(0, 0, box); qk_half(0, 1, box)
        qk_half(8, 0, box); qk_half(8, 1, box)
        vpsb_group()
        qk_half(1, 0, box); qk_half(1, 1, box)
        qk_half(9, 0, box); qk_half(9, 1, box)
        for tt in range(4):
            v_half(tt, 0, box); v_half(tt, 1, box)
        pwvp.close()   # frees wvp's 16KB for the attention pools below

        pexp = top.enter_context(tc.tile_pool(name="pexp", bufs=6))
        pep = top.enter_context(tc.tile_pool(name="pep", bufs=2))
        pbc = top.enter_context(tc.tile_pool(name="pbc", bufs=2))
        ptb = top.enter_context(tc.tile_pool(name="ptb", bufs=2))
        pua = top.enter_context(tc.tile_pool(name="pua", bufs=1))
        prs = top.enter_context(tc.tile_pool(name="prs", bufs=1))
        pstg = top.enter_context(tc.tile_pool(name="pstg", bufs=2))
        # The recip scratch rotates through one slot; its unused rows 1:32
        # flow through reciprocal+cast each stage, so pin them to 1.0 once
        # (recip(1)=1 keeps them finite forever; sel zeros them in the MM).
        rs_init = prs.tile([33, 1024], F32, tag="rs", name="rs_init")
        nc.vector.memset(rs_init, 1.0)

        # ---- stage loop ----
        stages = [(p, 0) for p in range(8)] + [(p, 1) for p in range(8)]
        prev = None
        for (p, ir) in stages:
            drain_until((p, ir))
            st = Stage(p, ir)
            st.front1()
            if prev is not None:
                prev.av_last()
                prev.combine_recip()
            st.front2()
            st.front2b()
            st.main(allow_op0=(ir == 1),
                    after_block=(prev.combine if prev is not None else None))
            prev = st
        prev.av_last()
        prev.combine_recip()
        b0 = [None]
        outproj_half(1, 0, 0, b0)   # k=0..3: PE cover for the final recip chain
        prev.combine()

        while pull_filler(True):
            pass

        # ---- tail: outproj hf1 ----
        outproj_half(1, 0, 1, b0)
        for m in range(1, 8):
            box = [None]
            outproj_half(1, m, 0, box)
            outproj_half(1, m, 1, box)

        if debug:
            for name, tile_ in (("d_qkT0", qkT[0]), ("d_qkT8", qkT[8]),
                                ("d_kpT0", kpT[0]), ("d_vsb0", vsb[0]),
                                ("d_vpsb", vpsb), ("d_yT0", yT[0]),
                                ("d_yT7", yT[7])):
                nc.sync.dma_start(out=dram[name].ap(), in_=tile_)


def _build():
    if "nc" in _CACHE:
        return _CACHE["nc"]
    import concourse.mybir as mybir
    import concourse.tile as tile
    from concourse import bacc

    BF = mybir.dt.bfloat16
    F32 = mybir.dt.float32
    nc = bacc.Bacc("TRN2", target_bir_lowering=False, debug=False,
                   enable_asserts=False)
    dram = {
        "xT": nc.dram_tensor("xT", [C, T], BF, kind="ExternalInput"),
        "pT": nc.dram_tensor("pT", [C, TP], BF, kind="ExternalInput"),
        "wqk": nc.dram_tensor("wqk", [C, 2 * C], BF, kind="ExternalInput"),
        "wv": nc.dram_tensor("wv", [C, C], BF, kind="ExternalInput"),
        "wkp": nc.dram_tensor("wkp", [C, C], BF, kind="ExternalInput"),
        "wvp": nc.dram_tensor("wvp", [C, C], BF, kind="ExternalInput"),
        "wp": nc.dram_tensor("wp", [C, C], BF, kind="ExternalInput"),
        "mask": nc.dram_tensor("mask", [128, 128], BF, kind="ExternalInput"),
        "maskp": nc.dram_tensor("maskp", [128, 64], BF, kind="ExternalInput"),
        "out": nc.dram_tensor("out", [C, T], F32, kind="ExternalOutput"),
    }
    with tile.TileContext(nc) as tc:
        _emit(nc, tc, dram)
    nc.compile()
    _CACHE["nc"] = nc
    return nc


def _host_consts():
    bf = ml_dtypes.bfloat16
    mask = np.triu(np.ones((128, 128), np.float32)).astype(bf)  # [key,query]=1 if q>=k
    tri = np.triu(np.ones((64, 64), np.float32))
    maskp = np.concatenate([tri, tri], axis=0).astype(bf)  # [128, 64]
    return mask, maskp


def _make_in_maps(x, prefix_embd, w_attn, w_prefix, w_proj):
    bf = ml_dtypes.bfloat16
    x = np.asarray(x, np.float32)
    prefix_embd = np.asarray(prefix_embd, np.float32)
    w_attn = np.asarray(w_attn, np.float32)
    w_prefix = np.asarray(w_prefix, np.float32)
    w_proj = np.asarray(w_proj, np.float32)
    mask, maskp = _host_consts()
    wqk = np.ascontiguousarray(w_attn[:, :2 * C]).astype(bf)
    wv = np.ascontiguousarray(w_attn[:, 2 * C:]).astype(bf)
    wkp = np.ascontiguousarray(w_prefix[:, C:2 * C]).astype(bf)
    wvp = np.ascontiguousarray(w_prefix[:, 2 * C:]).astype(bf)
    wp = w_proj.astype(bf)
    in_maps = []
    for i in range(B):
        in_maps.append({
            "xT": np.ascontiguousarray(x[i].T).astype(bf),
            "pT": np.ascontiguousarray(prefix_embd[i].T).astype(bf),
            "wqk": wqk, "wv": wv, "wkp": wkp, "wvp": wvp, "wp": wp,
            "mask": mask, "maskp": maskp,
        })
    return in_maps


def kernel(x, prefix_embd, w_attn, b_attn, w_prefix, b_prefix, w_proj, b_proj,
           **_ignored):
    nc = _build()
    in_maps = _make_in_maps(x, prefix_embd, w_attn, w_prefix, w_proj)
    from concourse.bass_utils import run_bass_kernel_spmd
    res = run_bass_kernel_spmd(nc, in_maps, core_ids=list(range(B)))
    out = np.stack([res.results[i]["out"].T for i in range(B)])
    return np.ascontiguousarray(out.astype(np.float32))


# revision 89
# speedup vs baseline: 1.0049x; 1.0049x over previous
"""Trainium2 Bass kernel for prefix-attention block (B=8,T=1024,C=1024,H=16,Tp=64).

Strategy: data-parallel over batch B across 8 NeuronCores (one batch element
per core, no collectives). Single fused software-pipelined schedule:

  - Inputs land via ~10 large multi-dim-AP DMAs (fast Sync issue, queues
    overlap with early compute).
  - Projections (q/k/v/prefix) are computed just-in-time and interleaved as
    "filler" PE work between attention score/AV blocks, so the TensorE never
    idles while ScalarE (exp) paces the attention inner loop.
  - Stage order: all (pair, ir=0) stages (queries 0:512), then all ir=1
    (queries 512:1024). Output projection for the first query half overlaps
    the second half's attention.
  - Per-stage combine: softmax sums come out of the AV matmul via a ones
    column; reciprocals are broadcast across partitions with a step-0-AP
    SBUF->SBUF DMA; normalization multiplies read the PSUM accumulators
    directly (no extraction copies, no tail combine phase).
"""

import numpy as np
import ml_dtypes

B, T, C, H, D, TP = 8, 1024, 1024, 16, 64, 64
NT = T // 128   # 8 token tiles
KC = C // 128   # 8 contraction chunks

_CACHE = {}


def _emit(nc, tc, dram, debug=False):
    import concourse.bass as bass
    import concourse.mybir as mybir
    from contextlib import ExitStack
    from concourse.tile_rust import add_dep_helper

    BF = mybir.dt.bfloat16
    F32 = mybir.dt.float32
    Exp = mybir.ActivationFunctionType.Exp

    pe_prev = [None]

    def pe_chain(inst):
        if pe_prev[0] is not None:
            add_dep_helper(inst.ins, pe_prev[0].ins, sync=False,
                           reason="forced PE order")
        pe_prev[0] = inst

    with ExitStack() as top:
        top.enter_context(nc.allow_low_precision(
            reason="bf16 compute is intentional; f32 PSUM accumulation"))
        persist = top.enter_context(tc.tile_pool(name="persist", bufs=1))
        ps_gen = top.enter_context(tc.tile_pool(name="ps_gen", bufs=2, space="PSUM"))
        ps_acc = top.enter_context(tc.tile_pool(name="ps_acc", bufs=2, space="PSUM"))

        # ---- persistent SBUF ----
        xT = persist.tile([128, KC * T], BF, tag="xT", name="xT")
        wqk = persist.tile([128, 16 * C], BF, tag="wqk", name="wqk")
        wv = persist.tile([128, KC * C], BF, tag="wv", name="wv")
        wp = persist.tile([128, 8 * C], BF, tag="wp", name="wp")
        pT = persist.tile([128, KC * TP], BF, tag="pT", name="pT")
        qkT = [persist.tile([128, T], BF, tag=f"qkT{m}", name=f"qkT{m}")
               for m in range(16)]
        vsb = [persist.tile([128, H * 65], BF, tag=f"vsb{t}", name=f"vsb{t}")
               for t in range(NT)]
        kpT = [persist.tile([128, TP], BF, tag=f"kpT{m}", name=f"kpT{m}")
               for m in range(8)]
        vpsb = persist.tile([128, H * 65], BF, tag="vpsb", name="vpsb")
        masksb = persist.tile([128, 128], BF, tag="masksb", name="masksb")
        maskpsb = persist.tile([128, 64], BF, tag="maskpsb", name="maskpsb")
        yT = [persist.tile([128, T], BF, tag=f"yT{t}", name=f"yT{t}")
              for t in range(NT)]

        # ---- input DMAs (few big ones; issue order = need order) ----
        nc.sync.dma_start(out=masksb, in_=dram["mask"].ap())
        nc.sync.dma_start(out=maskpsb, in_=dram["maskp"].ap())
        nc.sync.dma_start(out=pT.rearrange("p (k t) -> p k t", t=TP),
                          in_=dram["pT"].ap().rearrange("(k p) t -> p k t", p=128))

        def w_sliced_dma(dst, dram_t, col0, m_dst):
            # dst[:, (m_dst*KC + k)*128 : +128] = W[k*128:(k+1)*128, col0 : +128]
            dt = dram_t.ap()
            src = bass.AP(tensor=dt.tensor, offset=dt.offset + col0,
                          ap=[[dt.ap[0][0], 128],          # p within chunk
                              [dt.ap[0][0] * 128, KC],     # k
                              [1, 128]])                   # col
            dstv = bass.AP(tensor=dst.tensor,
                           offset=dst.offset + m_dst * KC * 128,
                           ap=[[dst.ap[0][0], 128],
                               [128, KC],
                               [1, 128]])
            nc.sync.dma_start(out=dstv, in_=src)

        wkp = persist.tile([128, 8 * C], BF, tag="wkp", name="wkp")
        pwvp = ExitStack()
        wvp = pwvp.enter_context(tc.tile_pool(name="pwvp", bufs=1)).tile(
            [128, KC * C], BF, tag="wvp", name="wvp")

        for m in range(2):
            w_sliced_dma(wkp, dram["wkp"], m * 128, m)
        for k2 in range(4):
            nc.sync.dma_start(
                out=xT.rearrange("p (k t) -> p k t", t=T)[:, 2 * k2:2 * k2 + 2, :],
                in_=dram["xT"].ap().rearrange("(k p) t -> p k t", p=128)
                [:, 2 * k2:2 * k2 + 2, :])
        w_sliced_dma(wqk, dram["wqk"], 0, 0)            # q pair 0
        w_sliced_dma(wqk, dram["wqk"], C, 8)            # k pair 0
        w_sliced_dma(wqk, dram["wqk"], 128, 1)          # q pair 1
        w_sliced_dma(wqk, dram["wqk"], C + 128, 9)      # k pair 1
        nc.sync.dma_start(out=wvp.rearrange("p (k c) -> p k c", c=C),
                          in_=dram["wvp"].ap().rearrange("(k p) c -> p k c", p=128))
        for k2 in range(2):
            nc.sync.dma_start(
                out=wv.rearrange("p (k c) -> p k c", c=C)[:, 4 * k2:4 * k2 + 4, :],
                in_=dram["wv"].ap().rearrange("(k p) c -> p k c", p=128)
                [:, 4 * k2:4 * k2 + 4, :])
        for m in range(2, 8):
            w_sliced_dma(wkp, dram["wkp"], m * 128, m)
            w_sliced_dma(wqk, dram["wqk"], m * 128, m)
            w_sliced_dma(wqk, dram["wqk"], C + m * 128, 8 + m)
        for m in range(8):
            w_sliced_dma(wp, dram["wp"], m * 128, m)

        def wqk_s(m, k):
            return wqk[:, (m * KC + k) * 128:(m * KC + k) * 128 + 128]

        def wkp_s(m, k):
            return wkp[:, (m * KC + k) * 128:(m * KC + k) * 128 + 128]

        def wp_s(m, k):
            return wp[:, (m * KC + k) * 128:(m * KC + k) * 128 + 128]

        def xT_s(k, sl=None):
            base = xT[:, k * T:(k + 1) * T]
            return base if sl is None else base[:, sl]

        def wv_s(k):
            return wv[:, k * C:(k + 1) * C]

        def wvp_s(k):
            return wvp[:, k * C:(k + 1) * C]

        def pT_s(k):
            return pT[:, k * TP:(k + 1) * TP]

        # ---- projection emitters (granular, for filler interleaving) ----
        def kpT_group(m):
            ps = ps_gen.tile([128, TP], F32, tag="ps_g", name="ps_g")
            for k in range(KC):
                pe_chain(nc.tensor.matmul(ps, wkp_s(m, k), pT_s(k),
                                          start=(k == 0), stop=(k == KC - 1)))
            nc.vector.tensor_copy(kpT[m], ps)

        def qk_half(m, hf, _box):
            ps = ps_gen.tile([128, 512], F32, tag="ps_g", name="ps_g")
            for k in range(KC):
                pe_chain(nc.tensor.matmul(
                    ps, wqk_s(m, k), xT_s(k)[:, hf * 512:(hf + 1) * 512],
                    start=(k == 0), stop=(k == KC - 1)))
            nc.vector.tensor_copy(qkT[m][:, hf * 512:(hf + 1) * 512], ps)

        def v_half(tt, hf, _box):
            ps = ps_gen.tile([128, 512], F32, tag="ps_g", name="ps_g")
            sl = slice(tt * 128, (tt + 1) * 128)
            for k in range(KC):
                pe_chain(nc.tensor.matmul(
                    ps, xT_s(k, sl), wv_s(k)[:, hf * 512:(hf + 1) * 512],
                    start=(k == 0), stop=(k == KC - 1)))
            nc.vector.tensor_copy(
                vsb[tt].rearrange("p (h e) -> p h e", e=65)
                [:, hf * 8:(hf + 1) * 8, 0:64],
                ps.rearrange("p (h e) -> p h e", e=64))
            if hf == 1:
                nc.vector.memset(
                    vsb[tt].rearrange("p (h e) -> p h e", e=65)[:, :, 64:65], 1.0)

        def vpsb_group():
            vpv = vpsb.rearrange("p (h e) -> p h e", e=65)
            for hf in range(2):
                ps = ps_gen.tile([64, 512], F32, tag="ps_g", name="ps_g")
                for k in range(KC):
                    pe_chain(nc.tensor.matmul(
                        ps, pT_s(k), wvp_s(k)[:, hf * 512:(hf + 1) * 512],
                        start=(k == 0), stop=(k == KC - 1)))
                nc.vector.tensor_copy(vpv[0:64, hf * 8:(hf + 1) * 8, 0:64],
                                      ps.rearrange("p (h e) -> p h e", e=64))
                nc.vector.tensor_copy(vpv[64:128, hf * 8:(hf + 1) * 8, 0:64],
                                      ps.rearrange("p (h e) -> p h e", e=64))
            nc.vector.memset(vpv[:, :, 64:65], 1.0)

        # outproj: chunk (hf, m) = sum_k wp[k,m-slice].T @ yT[k][:, hf*512:...]
        def outproj_half(hf, m, half, ps_box):
            if half == 0:
                ps_box[0] = ps_gen.tile([128, 512], F32, tag="ps_g", name="ps_g")
            ps = ps_box[0]
            for k in range(4 * half, 4 * half + 4):
                pe_chain(nc.tensor.matmul(
                    ps, wp_s(m, k), yT[k][:, hf * 512:(hf + 1) * 512],
                    start=(k == 0), stop=(k == KC - 1)))
            if half == 1:
                stg = pstg.tile([128, 512], F32, tag="stg", name="stg")
                if m % 2:
                    nc.vector.tensor_copy(stg, ps)
                else:
                    nc.scalar.copy(stg, ps)
                nc.sync.dma_start(
                    out=dram["out"].ap()[m * 128:(m + 1) * 128,
                                         hf * 512:(hf + 1) * 512],
                    in_=stg)
                ps_box[0] = None

        # ---- filler queue ----
        # Ordered list of (key, closure) emitted into PE idle slots; before a
        # stage starts, everything tagged with its key is force-drained so the
        # forced PE order can never deadlock against a data dependency.
        fillers = []

        def add_group(key, fn, nargs):
            box = [None]
            fillers.append((key, lambda: fn(*nargs, 0, box)))
            fillers.append((key, lambda: fn(*nargs, 1, box)))

        # pre-loop leaves these to fillers: kpT 2..7, qk pairs 2..7, v tt4..7
        for p in range(2, 8):
            fillers.append(((p, 0), lambda m=p: kpT_group(m)))
            add_group((p, 0), qk_half, (p,))
            add_group((p, 0), qk_half, (8 + p,))
            if p - 2 < 4:
                add_group((0, 1), v_half, (p + 2,))
        for m in range(8):
            add_group("op0", outproj_half, (0, m))

        def pull_filler(allow_op0):
            while fillers:
                key, f = fillers[0]
                if key == "op0" and not allow_op0:
                    return False
                fillers.pop(0)
                f()
                return True
            return False

        def drain_until(stage_key):
            while any(k == stage_key for k, _ in fillers):
                key, f = fillers.pop(0)
                f()

        # select matrix for the recip-broadcast matmuls: one K=33 matmul maps
        # scratch row 32 (A recips) -> out rows 0:64 and row 0 (B) -> 64:128
        sel = persist.tile([33, 128], BF, tag="sel", name="sel")
        nc.vector.memset(sel, 0.0)
        nc.vector.memset(sel[32:33, 0:64], 1.0)
        nc.vector.memset(sel[0:1, 64:128], 1.0)

        class Stage:
            def __init__(self, p, ir):
                self.p, self.ir = p, ir
                self.i0 = ir * 512
                self.jmax = 4 * (ir + 1)
                self.qt, self.kt, self.kpt = qkT[p], qkT[8 + p], kpT[p]
                self.s_all, self.e_all = {}, {}

            def scores(self, jb):
                c0 = max(0, jb - 4 * self.ir) * 128
                st = ps_gen.tile([128, 1024], F32, tag="ps_g", name="ps_g")
                for hh, pb in enumerate((0, 64)):
                    pe_chain(nc.tensor.matmul(
                        st[:, hh * 512 + c0:hh * 512 + 512],
                        self.kt[pb:pb + 64, jb * 128:(jb + 1) * 128],
                        self.qt[pb:pb + 64, self.i0 + c0:self.i0 + 512],
                        start=True, stop=True))
                self.s_all[jb] = st

            def exps(self, jb):
                c0 = max(0, jb - 4 * self.ir) * 128
                st = self.s_all.pop(jb)
                et = pexp.tile([128, 1024], BF, tag="et", name="et")
                nc.scalar.activation(
                    et.rearrange("p (g n) -> p g n", g=2)[:, :, c0:512],
                    st.rearrange("p (g n) -> p g n", g=2)[:, :, c0:512],
                    Exp, scale=0.125)
                if jb >= 4 * self.ir:
                    dv = et.rearrange("p (g n) -> p g n", g=2)[:, :, c0:c0 + 128]
                    nc.gpsimd.tensor_mul(
                        dv, dv,
                        bass.AP(tensor=masksb.tensor, offset=masksb.offset,
                                ap=[list(masksb.ap[0]), [0, 2],
                                    list(masksb.ap[1])]))
                self.e_all[jb] = et

            def avs(self, jb):
                c0 = max(0, jb - 4 * self.ir) * 128
                et = self.e_all.pop(jb)
                for hh in range(2):
                    h = 2 * self.p + hh
                    pe_chain(nc.tensor.matmul(
                        self.Ats[:, hh * 512 + c0:hh * 512 + 512],
                        vsb[jb][:, h * 65:(h + 1) * 65],
                        et[:, hh * 512 + c0:hh * 512 + 512],
                        start=(jb == 0), stop=(jb == self.jmax - 1),
                        skip_group_check=True))

            def front1(self):
                # prefix scores, both heads quadrant-packed into [128, 512]
                spt = ps_gen.tile([128, 512], F32, tag="ps_g", name="ps_g")
                for hh, pb in enumerate((0, 64)):
                    pe_chain(nc.tensor.matmul(
                        spt[pb:pb + 64, :],
                        self.kpt[pb:pb + 64, :],
                        self.qt[pb:pb + 64, self.i0:self.i0 + 512],
                        start=True, stop=True,
                        tile_position=(pb, pb)))
                self.scores(0)
                ep = pep.tile([128, 512], BF, tag="ep", name="ep")
                nc.scalar.activation(ep, spt, Exp, scale=0.125)
                if self.ir == 0:
                    nc.gpsimd.tensor_mul(ep[:, 0:64], ep[:, 0:64], maskpsb)
                if debug and self.p == 0 and self.ir == 0:
                    nc.sync.dma_start(out=dram["d_ep"].ap(), in_=ep)
                self.eps = ep
                self.exps(0)

            def front2(self):
                self.scores(1)
                self.Bts = ps_acc.tile([65, 1024], F32, tag="ps_a", name="ps_a")
                for hh, pb in enumerate((0, 64)):
                    h = 2 * self.p + hh
                    pe_chain(nc.tensor.matmul(
                        self.Bts[:, hh * 512:hh * 512 + 512],
                        vpsb[pb:pb + 64, h * 65:(h + 1) * 65],
                        self.eps[pb:pb + 64, :],
                        start=True, stop=True))
                self.exps(1)

            def front2b(self):
                # evacuate B early: unnormalized data to SBUF (DVE), sums row
                # to the recip scratch (ACT). Frees the Bts slot mid-stage.
                self.rs = prs.tile([33, 1024], F32, tag="rs", name="rs")
                nc.scalar.copy(self.rs[0:1, :], self.Bts[64:65, :])
                self.tB = ptb.tile([128, 1024], BF, tag="tB", name="tB")
                nc.vector.tensor_copy(self.tB[64:128, :], self.Bts[0:64, :])
                self.Ats = ps_acc.tile([65, 1024], F32, tag="ps_a", name="ps_a")

            def main(self, allow_op0, after_block=None):
                for jb0 in range(0, self.jmax, 2):
                    for jb in (jb0 + 2, jb0 + 3):
                        if jb < self.jmax:
                            self.scores(jb)
                    for jb in (jb0 + 2, jb0 + 3):
                        if jb < self.jmax:
                            self.exps(jb)
                    pull_filler(allow_op0)
                    if after_block is not None:
                        # prev stage's combine: its bc matmuls must precede
                        # avs(0) in the forced PE order (Ats slot rotation)
                        after_block()
                        after_block = None
                        pull_filler(allow_op0)  # PE cover for the evac+mul
                    for jb in (jb0, jb0 + 1):
                        if jb < self.jmax - 1:
                            self.avs(jb)

            def av_last(self):
                self.avs(self.jmax - 1)

            def combine_recip(self):
                # A sums -> scratch row 32 (ACT), batched reciprocal, cast.
                # Off the forced-PE path so the PE keeps streaming meanwhile.
                rs = self.rs
                nc.scalar.copy(rs[32:33, :], self.Ats[64:65, :])
                nc.vector.reciprocal_approx_fast(rs, rs)
                self.rsb = prs.tile([33, 1024], BF, tag="rsb", name="rsb")
                nc.vector.tensor_copy(self.rsb, rs)

            def combine(self):
                # K=1 ones-matmul broadcast into PSUM (A rows 0:64, B rows
                # 64:128), evac to bf16 SBUF, then normalize+sum into yT.
                rsb = self.rsb
                bc_ps = ps_gen.tile([128, 1024], F32, tag="ps_g", name="ps_g")
                for hh in range(2):
                    cs = slice(hh * 512, (hh + 1) * 512)
                    pe_chain(nc.tensor.matmul(          # A -> rows 0:64
                        bc_ps[0:64, cs], sel[32:33, 0:64], rsb[32:33, cs],
                        start=True, stop=True, tile_position=(32, 0)))
                    pe_chain(nc.tensor.matmul(          # B -> rows 64:128
                        bc_ps[64:128, cs], sel[0:1, 64:128], rsb[0:1, cs],
                        start=True, stop=True, tile_position=(0, 64)))
                bc = pbc.tile([128, 1024], BF, tag="bc", name="bc")
                nc.scalar.copy(bc, bc_ps)
                uA = pua.tile([64, 1024], BF, tag="uA", name="uA")
                nc.vector.tensor_mul(uA, self.Ats[0:64, :], bc[0:64, :])
                uB = pua.tile([64, 1024], BF, tag="uB", name="uB")
                nc.vector.tensor_mul(uB, self.tB[64:128, :], bc[64:128, :])
                if debug and self.p == 0 and self.ir == 0:
                    nc.sync.dma_start(out=dram["d_bcA"].ap(), in_=bc[0:64, :])
                    nc.sync.dma_start(out=dram["d_uA"].ap(), in_=uA)
                    nc.sync.dma_start(out=dram["d_tB"].ap(), in_=uB)
                    nc.sync.dma_start(out=dram["d_bcB"].ap(), in_=bc[64:128, :])
                sl = slice(self.i0, self.i0 + 512)
                nc.vector.tensor_add(yT[self.p][0:64, sl],
                                     uA[:, 0:512], uB[:, 0:512])
                nc.vector.tensor_add(yT[self.p][64:128, sl],
                                     uA[:, 512:1024], uB[:, 512:1024])

        # ---- pre-loop: minimum to start stage (0, ir=0) ----
        box = [None]
        kpT_group(0)
        kpT_group(1)
        qk_half(0, 0, box); qk_half(0, 1, box)
        qk_half(8, 0, box); qk_half(8, 1, box)
        qk_half(1, 0, box); qk_half(1, 1, box)
        qk_half(9, 0, box); qk_half(9, 1, box)
        vpsb_group()
        for tt in range(4):
            v_half(tt, 0, box); v_half(tt, 1, box)
        pwvp.close()   # frees wvp's 16KB for the attention pools below

        pexp = top.enter_context(tc.tile_pool(name="pexp", bufs=6))
        pep = top.enter_context(tc.tile_pool(name="pep", bufs=2))
        pbc = top.enter_context(tc.tile_pool(name="pbc", bufs=2))
        ptb = top.enter_context(tc.tile_pool(name="ptb", bufs=2))
        pua = top.enter_context(tc.tile_pool(name="pua", bufs=1))
        prs = top.enter_context(tc.tile_pool(name="prs", bufs=1))
        pstg = top.enter_context(tc.tile_pool(name="pstg", bufs=2))
        # The recip scratch rotates through one slot; its unused rows 1:32
        # flow through reciprocal+cast each stage, so pin them to 1.0 once
        # (recip(1)=1 keeps them finite forever; sel zeros them in the MM).
        rs_init = prs.tile([33, 1024], F32, tag="rs", name="rs_init")
        nc.vector.memset(rs_init, 1.0)

        # ---- stage loop ----
        stages = [(p, 0) for p in range(8)] + [(p, 1) for p in range(8)]
        prev = None
        for (p, ir) in stages:
            drain_until((p, ir))
            st = Stage(p, ir)
            st.front1()
            if prev is not None:
                prev.av_last()
                prev.combine_recip()
            st.front2()
            st.front2b()
            st.main(allow_op0=(ir == 1),
                    after_block=(prev.combine if prev is not None else None))
            prev = st
        prev.av_last()
        prev.combine_recip()
        prev.combine()

        while pull_filler(True):
            pass

        # ---- tail: outproj hf1 ----
        for m in range(8):
            box = [None]
            outproj_half(1, m, 0, box)
            outproj_half(1, m, 1, box)

        if debug:
            for name, tile_ in (("d_qkT0", qkT[0]), ("d_qkT8", qkT[8]),
                                ("d_kpT0", kpT[0]), ("d_vsb0", vsb[0]),
                                ("d_vpsb", vpsb), ("d_yT0", yT[0]),
                                ("d_yT7", yT[7])):
                nc.sync.dma_start(out=dram[name].ap(), in_=tile_)


def _build():
    if "nc" in _CACHE:
        return _CACHE["nc"]
    import concourse.mybir as mybir
    import concourse.tile as tile
    from concourse import bacc

    BF = mybir.dt.bfloat16
    F32 = mybir.dt.float32
    nc = bacc.Bacc("TRN2", target_bir_lowering=False, debug=False,
                   enable_asserts=False)
    dram = {
        "xT": nc.dram_tensor("xT", [C, T], BF, kind="ExternalInput"),
        "pT": nc.dram_tensor("pT", [C, TP], BF, kind="ExternalInput"),
        "wqk": nc.dram_tensor("wqk", [C, 2 * C], BF, kind="ExternalInput"),
        "wv": nc.dram_tensor("wv", [C, C], BF, kind="ExternalInput"),
        "wkp": nc.dram_tensor("wkp", [C, C], BF, kind="ExternalInput"),
        "wvp": nc.dram_tensor("wvp", [C, C], BF, kind="ExternalInput"),
        "wp": nc.dram_tensor("wp", [C, C], BF, kind="ExternalInput"),
        "mask": nc.dram_tensor("mask", [128, 128], BF, kind="ExternalInput"),
        "maskp": nc.dram_tensor("maskp", [128, 64], BF, kind="ExternalInput"),
        "out": nc.dram_tensor("out", [C, T], F32, kind="ExternalOutput"),
    }
    with tile.TileContext(nc) as tc:
        _emit(nc, tc, dram)
    nc.compile()
    _CACHE["nc"] = nc
    return nc


def _host_consts():
    bf = ml_dtypes.bfloat16
    mask = np.triu(np.ones((128, 128), np.float32)).astype(bf)  # [key,query]=1 if q>=k
    tri = np.triu(np.ones((64, 64), np.float32))
    maskp = np.concatenate([tri, tri], axis=0).astype(bf)  # [128, 64]
    return mask, maskp


def _make_in_maps(x, prefix_embd, w_attn, w_prefix, w_proj):
    bf = ml_dtypes.bfloat16
    x = np.asarray(x, np.float32)
    prefix_embd = np.asarray(prefix_embd, np.float32)
    w_attn = np.asarray(w_attn, np.float32)
    w_prefix = np.asarray(w_prefix, np.float32)
    w_proj = np.asarray(w_proj, np.float32)
    mask, maskp = _host_consts()
    wqk = np.ascontiguousarray(w_attn[:, :2 * C]).astype(bf)
    wv = np.ascontiguousarray(w_attn[:, 2 * C:]).astype(bf)
    wkp = np.ascontiguousarray(w_prefix[:, C:2 * C]).astype(bf)
    wvp = np.ascontiguousarray(w_prefix[:, 2 * C:]).astype(bf)
    wp = w_proj.astype(bf)
    in_maps = []
    for i in range(B):
        in_maps.append({
            "xT": np.ascontiguousarray(x[i].T).astype(bf),
            "pT": np.ascontiguousarray(prefix_embd[i].T).astype(bf),
            "wqk": wqk, "wv": wv, "wkp": wkp, "wvp": wvp, "wp": wp,
            "mask": mask, "maskp": maskp,
        })
    return in_maps


def kernel(x, prefix_embd, w_attn, b_attn, w_prefix, b_prefix, w_proj, b_proj,
           **_ignored):
    nc = _build()
    in_maps = _make_in_maps(x, prefix_embd, w_attn, w_prefix, w_proj)
    from concourse.bass_utils import run_bass_kernel_spmd
    res = run_bass_kernel_spmd(nc, in_maps, core_ids=list(range(B)))
    out = np.stack([res.results[i]["out"].T for i in range(B)])
    return np.ascontiguousarray(out.astype(np.float32))
